# revision 16
# baseline (speedup 1.0000x reference)
"""Trainium2 Bass kernel for leave-one-out Nadaraya-Watson regression
(nn_Net_41420664602632, retrieval_knn).

Math
----
reference:
    Fx = x @ W.T ; Ft = train_X @ W.T          [N, 3]
    K[j,i,c] = exp(-((Ft[j,c]-Fx[i,c])/h)^2/2), K[i,i,c] = 0
    out[i,c] = sum_j K[j,i,c]*Y[j,c] / sum_j K[j,i,c]

With a = Ft/(sqrt(2)*h), b = Fx/(sqrt(2)*h) this is, per channel, a 1-D
Gaussian kernel regression: out[i] = numt(b_i)/dent(b_i) with
    numt(t) = sum_j Y_j exp(-(t-a_j)^2),  dent(t) = sum_j exp(-(t-a_j)^2)
numt/dent are Gaussian-smoothed fields with fixed width 1 in t-space
(the 1/(sqrt(2)h) scaling normalizes the bandwidth away), so instead of
evaluating them at all N=4096 query points (O(N^2) pairwise exps), the
device evaluates them on a uniform T=64-point grid covering the query
range (O(N*T)), and the host Catmull-Rom-interpolates at the 4096 query
positions (grid step ~0.06 of data range << kernel width ~0.2 of it =>
total error ~4e-4 relative, dominated by the fp16 matmul operands, far
inside the 2e-2 gate; validated against the reference in numpy).

Device program (per core, j-shard of 512 training points)
---------------------------------------------------------
Hand-scheduled Bass (no TileContext): per-engine instruction queues with
manual semaphores. The TileContext scheduler adds ~0.3-0.7us of
semaphore bookkeeping around every instruction plus a ~5us exit sweep
that resets every allocated semaphore on every engine; with only ~50
real instructions this overhead dominated, so the program is wired by
hand (no buffer reuse -> no WAR hazards, 7 semaphores total).

The grid is an fp32 iota 0..T-1 (no DMA), and the affine grid transform
is folded into per-partition scalars computed on the host:
    arg[j,t] = (2 a_j dg) * t + (2 a_j lo - a_j^2)
Per j-tile: 3 tensor_scalar ops (split across DVE and GpSimd) build
arg[128, 3T], one ScalarE ACT takes exp of the whole tile into fp16,
and 3 single-pass fp16 PE matmuls [Y_j, 1]^T @ E accumulate num/den per
channel into distinct PSUM banks / PE col-groups.
Host sums the 8 cores' [2, 3T] partials, multiplies by exp(-g^2),
interpolates at b, subtracts the j==i self term, and divides.
"""

import numpy as np

from concourse import bacc, mybir
from concourse.bass_utils import run_bass_kernel_spmd

N = 4096       # training/query points
C = 3          # projected channels (fc1 out_features)
NCORES = 8
JSH = N // NCORES        # 512: j-shard per core
JTILES = JSH // 128      # 4
T = 64                   # grid targets

_CACHE = {}


def _build_nc(n=N, ncores=NCORES, t=T):
    key = (n, ncores, t)
    if key in _CACHE:
        return _CACHE[key]
    jtiles = (n // ncores) // 128
    f32 = mybir.dt.float32
    f16 = mybir.dt.float16
    Exp = mybir.ActivationFunctionType.Exp

    nc = bacc.Bacc("TRN2", target_bir_lowering=False, debug=False)
    # one [128, 128] f32 input (512B rows -> line-rate DMA):
    #   cols 2m/2m+1 (m = c*jtiles+jt): scale' = 2*a*dg, bias' = 2*a*lo-a^2
    #   col 64+m: (Y[j,c], 1.0) packed as two fp16 -> matmul lhsT via bitcast
    sbst_d = nc.dram_tensor("sbst", [128, 128], f32, kind="ExternalInput")
    out_d = nc.dram_tensor("out", [2, C * t], f32, kind="ExternalOutput")

    sbst = nc.alloc_sbuf_tensor("sbst_sb", [128, 128], f32)
    ramp = nc.alloc_sbuf_tensor("ramp_sb", [128, t], f32)
    args = nc.alloc_sbuf_tensor("args_sb", [128, jtiles * C * t], f32)
    gbuf = nc.alloc_sbuf_tensor("g_sb", [128, jtiles * C * t], f16)
    outsb = nc.alloc_sbuf_tensor("out_sb", [2, C * t], f32)
    acc = nc.alloc_psum_tensor("acc_ps", [128, 2048], f32)

    s_in = nc.alloc_semaphore("s_in")      # input DMA halves (+16 each)
    s_ramp = nc.alloc_semaphore("s_ramp")  # iota done
    s_argv = nc.alloc_semaphore("s_argv")  # DVE arg ops done
    s_argg = nc.alloc_semaphore("s_argg")  # GpSimd arg ops done
    s_g = nc.alloc_semaphore("s_g")        # exp tiles done
    s_mm = nc.alloc_semaphore("s_mm")      # per-channel accumulation closed
    s_ev = nc.alloc_semaphore("s_ev")      # evacuation copies done
    s_out = nc.alloc_semaphore("s_out")    # output DMA done

    # which engine computes arg (jt, c)
    def arg_eng(jt, c):
        return nc.gpsimd if c == 2 else nc.vector

    # cumulative arg-op counts per engine after each jt batch
    nv = [0] * jtiles
    ng = [0] * jtiles
    v = g = 0
    for jt in range(jtiles):
        for c in range(C):
            if arg_eng(jt, c) is nc.vector:
                v += 1
            else:
                g += 1
        nv[jt], ng[jt] = v, g

    aslc = lambda jt, c: args.ap()[:, (jt * C + c) * t : (jt * C + c + 1) * t]
    gslc = lambda jt, c: gbuf.ap()[:, (jt * C + c) * t : (jt * C + c + 1) * t]

    # --- sync: input DMA (descriptors spread across all 16 hw queues on
    # their own; one dma_start means one completion-signal latency), then
    # the final output DMA (walrus's NEFF epilogue drains the queues, so
    # no completion wait is needed) ---
    nc.sync.dma_start(sbst.ap()[:, 0:64], sbst_d.ap()[:, 0:64]).then_inc(s_in, 16)
    nc.sync.dma_start(sbst.ap()[:, 64:128], sbst_d.ap()[:, 64:128]).then_inc(
        s_in, 16
    )

    # --- scalar: exp-table warm, the 4 exp ACTs, evac c1 ---
    warm = nc.alloc_sbuf_tensor("warm_sb", [128, 1], f32)
    nc.scalar.activation(warm.ap(), nc.const_aps.scalar_like(0.0, warm.ap()), Exp)
    for jt in range(jtiles):
        nc.scalar.wait_ge(s_argv, nv[jt])
        nc.scalar.wait_ge(s_argg, ng[jt])
        nc.scalar.activation(
            gbuf.ap()[:, jt * C * t : (jt + 1) * C * t],
            args.ap()[:, jt * C * t : (jt + 1) * C * t],
            Exp,
        ).then_inc(s_g)
    nc.scalar.wait_ge(s_mm, 2)
    nc.scalar.copy(
        outsb.ap()[:, t : 2 * t], acc.ap()[32 : 32 + 2, 512 : 512 + t]
    ).then_inc(s_ev)

    # --- vector: 8 arg ops (c0/c1; DVE is ~1.6x faster per op than
    # GpSimd, so it carries two channels), evac c0 and c2 ---
    nc.vector.wait_ge(s_in, 16)
    nc.vector.wait_ge(s_ramp, 1)
    for jt in range(jtiles):
        for c in range(C):
            if arg_eng(jt, c) is nc.vector:
                k = 2 * (c * jtiles + jt)
                nc.vector.tensor_scalar(
                    aslc(jt, c),
                    ramp.ap(),
                    sbst.ap()[:, k : k + 1],
                    sbst.ap()[:, k + 1 : k + 2],
                    mybir.AluOpType.mult,
                    mybir.AluOpType.add,
                ).then_inc(s_argv)
    nc.vector.wait_ge(s_mm, 1)
    nc.vector.tensor_copy(
        outsb.ap()[:, 0:t], acc.ap()[0:2, 0:t]
    ).then_inc(s_ev)
    nc.vector.wait_ge(s_mm, 3)
    nc.vector.tensor_copy(
        outsb.ap()[:, 2 * t : 3 * t], acc.ap()[64 : 64 + 2, 1024 : 1024 + t]
    ).then_inc(s_ev)

    # --- gpsimd: iota ramp, 4 arg ops (c2), output DMA ---
    nc.gpsimd.iota(
        ramp.ap(), [[1, t]], channel_multiplier=0,
        allow_small_or_imprecise_dtypes=True,
    ).then_inc(s_ramp)
    nc.gpsimd.wait_ge(s_in, 16)
    for jt in range(jtiles):
        for c in range(C):
            if arg_eng(jt, c) is nc.gpsimd:
                k = 2 * (c * jtiles + jt)
                nc.gpsimd.tensor_scalar(
                    aslc(jt, c),
                    ramp.ap(),
                    sbst.ap()[:, k : k + 1],
                    sbst.ap()[:, k + 1 : k + 2],
                    mybir.AluOpType.mult,
                    mybir.AluOpType.add,
                ).then_inc(s_argg)
    nc.gpsimd.wait_ge(s_ev, C)
    nc.gpsimd.dma_start(out_d.ap(), outsb.ap()).then_inc(s_out, 16)

    # --- tensor: 12 fp16 matmuls, 3 channels on distinct col-groups ---
    nc.tensor.wait_ge(s_in, 32)
    for jt in range(jtiles):
        nc.tensor.wait_ge(s_g, jt + 1)
        for c in range(C):
            m = c * jtiles + jt
            mm = nc.tensor.matmul(
                acc.ap()[32 * c : 32 * c + 2, c * 512 : c * 512 + t],
                lhsT=sbst.ap()[:, 64 + m : 64 + m + 1].bitcast(f16),
                rhs=gslc(jt, c),
                start=(jt == 0),
                stop=(jt == jtiles - 1),
                tile_position=(0, 32 * c),
            )
            if jt == jtiles - 1:
                mm.then_inc(s_mm)

    nc.compile()
    _CACHE[key] = nc
    return nc


def _prep_inputs(x, train_X, Y, W, h, n=N, ncores=NCORES, t=T):
    """Host-side prep: projections, grid, per-core scale/bias maps."""
    jsh = n // ncores
    jtiles = jsh // 128
    x64 = np.asarray(x, np.float64)
    t64 = np.asarray(train_X, np.float64)
    W64 = np.asarray(W, np.float64)
    hv = float(np.asarray(h).reshape(-1)[0])
    s = 1.0 / (np.sqrt(2.0) * hv)
    b = (x64 @ W64.T) * s          # queries   [n, C]
    a = (t64 @ W64.T) * s          # training  [n, C]
    a32 = a.astype(np.float32)
    b32 = b.astype(np.float32)

    # uniform grid over the query range with a 2-step margin so every
    # query interpolates from an interior Catmull-Rom stencil
    minv = float(b32.min())
    maxv = float(b32.max())
    dg = (maxv - minv) / (t - 5) if maxv > minv else 1.0
    lo = minv - 2.0 * dg

    Yf = np.asarray(Y, np.float64).astype(np.float32)
    one16 = np.uint32(np.float16(1.0).view(np.uint16)) << np.uint32(16)

    in_maps = []
    for r in range(ncores):
        j0 = r * jsh
        m = np.zeros((128, 128), np.float32)
        mu = m.view(np.uint32)
        for c in range(C):
            for jt in range(jtiles):
                kk = c * jtiles + jt
                aj = a32[j0 + jt * 128 : j0 + (jt + 1) * 128, c].astype(np.float64)
                m[:, 2 * kk] = (2.0 * aj * dg).astype(np.float32)
                m[:, 2 * kk + 1] = (2.0 * aj * lo - aj * aj).astype(np.float32)
                y16 = Yf[j0 + jt * 128 : j0 + (jt + 1) * 128, c].astype(np.float16)
                mu[:, 64 + kk] = y16.view(np.uint16).astype(np.uint32) | one16
        in_maps.append({"sbst": m})
    return in_maps, a32, b32, lo, dg


def _interp_catmull_rom(f, lo, dg, xq, t=T):
    """Cubic Catmull-Rom interpolation of f (uniform grid) at xq."""
    u = (np.asarray(xq, np.float64) - lo) / dg
    i = np.clip(np.floor(u).astype(np.int64), 1, t - 3)
    u = u - i
    fm1, f0, f1, f2 = f[i - 1], f[i], f[i + 1], f[i + 2]
    return 0.5 * (
        2.0 * f0
        + (f1 - fm1) * u
        + (2.0 * fm1 - 5.0 * f0 + 4.0 * f1 - f2) * u * u
        + (3.0 * f0 - fm1 - 3.0 * f1 + f2) * u * u * u
    )


def _combine(results, Y, a32, b32, lo, dg, n=N, t=T):
    """Sum per-core partials, damp, interpolate, self-subtract, divide."""
    num = np.zeros((C, t), np.float64)
    den = np.zeros((C, t), np.float64)
    for res in results:
        o = np.asarray(res["out"], np.float64)  # [2, C*t]
        num += o[0].reshape(C, t)
        den += o[1].reshape(C, t)
    grid = lo + dg * np.arange(t, dtype=np.float64)
    damp = np.exp(-(grid**2))
    num *= damp
    den *= damp

    Yf = np.asarray(Y, np.float64)
    out = np.empty((n, C), np.float64)
    for c in range(C):
        ni = _interp_catmull_rom(num[c], lo, dg, b32[:, c], t)
        di = _interp_catmull_rom(den[c], lo, dg, b32[:, c], t)
        # leave-one-out: remove the j == i term exp(-(b_i - a_i)^2)
        kii = np.exp(
            -((b32[:, c].astype(np.float64) - a32[:, c].astype(np.float64)) ** 2)
        )
        out[:, c] = (ni - kii * Yf[:, c]) / (di - kii)
    return out.astype(np.float32)


def kernel(x, train_X, Y, W, h):
    nc = _build_nc()
    in_maps, a32, b32, lo, dg = _prep_inputs(x, train_X, Y, W, h)
    res = run_bass_kernel_spmd(nc, in_maps, core_ids=list(range(NCORES)))
    return _combine(res.results, Y, a32, b32, lo, dg)
